# revision 40
# baseline (speedup 1.0000x reference)
"""Multi-head attention Trainium2 kernel (B=4, N=2048, D=1024, H=16).

Sharding: 8 cores = 4 batches x 2 head-groups (8 heads each), zero
collectives. Each core:
  - all projections in fp16, interleaved into the attention pipeline so
    the scalar engine (exp) starts ~10us in and stays saturated:
    x arrives as 512-column slices; the first head-pair's k-projection and
    q-projection run as the slices land, its S matmuls follow immediately,
    and the v-projection row-blocks are emitted just-in-time before the
    PV matmul that consumes them
  - q,k kept transposed [feat, seq]; v row-layout, augmented with a ones
    column so the PV matmul emits the softmax denominator for free
  - attention per head-pair x 512-query chunk: S matmuls packed two heads
    per pass via disjoint PE row groups into one [128,1024] PSUM tile,
    one wide exp on ACT (scale=1/8, fp16 out), PV accumulation with
    128-column stationary windows (fast-weight-load path)
  - software pipeline: each unit's S phase is split 8/8 around the
    previous unit's PV loop (exp pool holds up to 34 tiles) so the scalar
    engine never starves; projection/out-projection/normalization work
    fills the PE slack inside the ACT-paced PV loops; the last chunk's
    out-projection is pre-accumulated over pairs 0..2 during the final
    unit and finished right after its norm
  - normalization: one 64-wide ones matmul per head broadcasts the fp16
    denominator across partitions, one reciprocal_approx_fast and one
    multiply produce the normalized [128, 512] fp16 tile per unit
  - out-projection partial [1024,2048] per chunk, evacuated via DVE
Host sums the two head-group partials per batch and adds bias.
"""
from collections import deque
from contextlib import ExitStack

import numpy as np

import concourse.mybir as mybir
import concourse.tile as tile
from concourse import bacc
from concourse.bass_utils import run_bass_kernel_spmd

F32 = mybir.dt.float32
F16 = mybir.dt.float16

P = 128
N = 2048         # sequence length
DI = 1024        # model dim
NH = 8           # heads per core
HD = 64          # head dim
NPAIR = 4        # head pairs per core
KT = 8           # contraction tiles for projections
CH = 512         # query chunk width
NCHUNK = 4       # chunks per sequence
MT = 16          # key tiles (m) per sequence
ET = 8           # output-feature blocks
SCALE = HD ** -0.5
VW = HD + 1      # v columns per head incl. denominator ones-column
VFLAT = MT * NH * VW

_NC_CACHE = None


def _build():
    nc = bacc.Bacc("TRN2", target_bir_lowering=False, debug=False)

    xT = nc.dram_tensor("xT", [DI, N], F16, kind="ExternalInput").ap()
    wqkA = nc.dram_tensor("wqkA", [8, P, KT, P], F16, kind="ExternalInput").ap()
    wvA = nc.dram_tensor("wvA", [P, KT, 512], F16, kind="ExternalInput").ap()
    woT = nc.dram_tensor("woT", [512, DI], F16, kind="ExternalInput").ap()
    cstd = nc.dram_tensor("cst", [P, 129], F16, kind="ExternalInput").ap()
    outT = nc.dram_tensor("outT", [DI, N], F16, kind="ExternalOutput").ap()

    xT_r = xT.rearrange("(k p) n -> k p n", p=P)        # [8, 128, 2048]
    woT_r = woT.rearrange("(k p) e -> k p e", p=P)      # [4, 128, 1024]
    outT_r = outT.rearrange("(e p) n -> e p n", p=P)    # [8, 128, 2048]

    with tile.TileContext(nc) as tc, ExitStack() as persist:
        qk_pool = persist.enter_context(tc.tile_pool(name="qkp", bufs=4))
        va_pool = persist.enter_context(tc.tile_pool(name="vap", bufs=1))
        misc = persist.enter_context(tc.tile_pool(name="misc", bufs=1))
        wqk_pool = persist.enter_context(tc.tile_pool(name="wqk", bufs=2))
        xt_pool = persist.enter_context(tc.tile_pool(name="xt", bufs=8))
        wv_pool = persist.enter_context(tc.tile_pool(name="wv", bufs=1))
        wq_pool = persist.enter_context(tc.tile_pool(name="wq", bufs=4))
        wo_pool = persist.enter_context(tc.tile_pool(name="wo", bufs=4))
        exp_pool = persist.enter_context(tc.tile_pool(name="expp", bufs=36))
        ot_pool = persist.enter_context(tc.tile_pool(name="ot", bufs=8))
        osb_pool = persist.enter_context(tc.tile_pool(name="osb", bufs=4))
        stage_pool = persist.enter_context(tc.tile_pool(name="stg", bufs=6))
        den_pool = persist.enter_context(tc.tile_pool(name="den", bufs=8))
        rbc_pool = persist.enter_context(tc.tile_pool(name="rbc", bufs=2))
        sps_pool = persist.enter_context(
            tc.tile_pool(name="sps", bufs=2, space="PSUM"))
        oaug_pool = persist.enter_context(
            tc.tile_pool(name="oaug", bufs=2, space="PSUM"))
        aux_pool = persist.enter_context(
            tc.tile_pool(name="aux", bufs=2, space="PSUM"))

        # --- input DMAs, ordered so the pipeline lights up ASAP: the
        # first k-projection group needs wqk[4] + the chunk-0 columns of
        # every x k-tile; q needs wq0; v needs wv. Early loads are split
        # across the two HWDGE rings (sync + scalar) so the startup
        # transfers run in parallel; the scalar ring is only used before
        # the first exp is queued, so the triggers never stall ACT.
        wqk_first = wqk_pool.tile([P, KT, P], F16, tag="wqk")
        nc.sync.dma_start(wqk_first[:], wqkA[4])
        cst = misc.tile([P, 129], F16)
        xt = [xt_pool.tile([P, N], F16, name=f"xt{k}", tag="xt")
              for k in range(KT)]
        wq = [wq_pool.tile([P, KT, P], F16, name=f"wq{f}", tag="wq")
              for f in range(4)]
        # x arrives in 1024-column halves: per-partition lines are 2KB
        # instead of 1KB, halving the packet count for the same data — the
        # cold DMA phase is packet-count-bound, so chunks 0+1 land in the
        # time chunk 0 alone used to take
        for k in range(KT):
            nc.sync.dma_start(xt[k][:, 0:2 * CH], xT_r[k][:, 0:2 * CH])
        nc.sync.dma_start(wq[0][:], wqkA[0])
        nc.sync.dma_start(cst[:], cstd[:])
        wv = wv_pool.tile([P, KT, 512], F16)
        nc.sync.dma_start(wv[:], wvA[:])
        wqk_tiles = {4: wqk_first}
        nc.sync.dma_start(wq[1][:], wqkA[1])
        for k in range(KT):
            nc.sync.dma_start(xt[k][:, 2 * CH:N], xT_r[k][:, 2 * CH:N])
        nc.sync.dma_start(wq[2][:], wqkA[2])
        nc.sync.dma_start(wq[3][:], wqkA[3])
        wo = [wo_pool.tile([P, DI], F16, name=f"wo{kk}", tag="wo")
              for kk in range(NPAIR)]
        for kk in range(NPAIR):
            nc.sync.dma_start(wo[kk][:], woT_r[kk])

        # k tiles persist (full sequence, read by every chunk); q chunks
        # rotate through a small pool — each q(c,p) is dead once unit (c,p)'s
        # S phase completes, so 6 buffers cover the pipeline depth
        kT = [qk_pool.tile([P, N], F16, name=f"kT{t}", tag="qkT")
              for t in range(4)]
        qc_pool = persist.enter_context(tc.tile_pool(name="qc", bufs=6))
        qc_map = {}
        va_t = va_pool.tile([P, VFLAT + 64], F16)
        nc.vector.memset(va_t[:, VFLAT:VFLAT + 64], 0.0)
        v_aug = va_t[:, 0:VFLAT].rearrange("p (m h d) -> p m h d", h=NH, d=VW)
        nc.vector.tensor_copy(v_aug[:, :, :, HD:HD + 1],
                              cst[:, 0:1].to_broadcast((P, MT, NH, 1)))

        # ---- emission helpers -------------------------------------------
        # Projection chains accumulate 8 k-tiles into one PSUM tile.
        # Back-to-back accumulates into the same PSUM region pay a ~210ns
        # RAW hazard (the previous matmul's PSUM writes must retire), so
        # every chain is emitted as per-k "steps" that get woven between
        # other matmuls — either another chain (weave/pairs) or the PV
        # matmuls of the surrounding sweep (fill slots).
        kdone = set()

        def kproj_stepper(p, cc):
            if (p, cc) in kdone:
                return []
            kdone.add((p, cc))
            f = 4 + p
            if f not in wqk_tiles:
                t = wqk_pool.tile([P, KT, P], F16, tag="wqk")
                nc.sync.dma_start(t[:], wqkA[f])
                wqk_tiles[f] = t
            st = {}
            csl = slice(cc * CH, (cc + 1) * CH)

            def step(k):
                if 'ps' not in st:
                    st['ps'] = aux_pool.tile([P, CH], F32, tag="aux",
                                             name=f"kp_{p}_{cc}")
                nc.tensor.matmul(st['ps'][:], wqk_tiles[f][:, k, :],
                                 xt[k][:, csl],
                                 start=(k == 0), stop=(k == KT - 1))
                if k == KT - 1:
                    nc.vector.tensor_copy(kT[p][:, csl], st['ps'][:])
            return [lambda k=k: step(k) for k in range(KT)]

        qp_started = set()

        def qproj_stepper(c, p):
            if (c, p) in qc_map or (c, p) in qp_started:
                return []
            qp_started.add((c, p))
            st = {}
            csl = slice(c * CH, (c + 1) * CH)

            def step(k):
                if 'ps' not in st:
                    st['ps'] = aux_pool.tile([P, CH], F32, tag="aux",
                                             name=f"qp_{c}_{p}")
                nc.tensor.matmul(st['ps'][:], wq[p][:, k, :], xt[k][:, csl],
                                 start=(k == 0), stop=(k == KT - 1))
                if k == KT - 1:
                    qc = qc_pool.tile([P, CH], F16, name=f"qc_{c}_{p}",
                                      tag="qc")
                    nc.vector.tensor_copy(qc[:], st['ps'][:])
                    qc_map[(c, p)] = qc
            return [lambda k=k: step(k) for k in range(KT)]

        def vproj_stepper(r):
            st = {}

            def step(k):
                if 'ps' not in st:
                    st['ps'] = aux_pool.tile([P, CH], F32, tag="aux",
                                             name=f"vp_{r}")
                nc.tensor.matmul(st['ps'][:], xt[k][:, r * P:(r + 1) * P],
                                 wv[:, k, :],
                                 start=(k == 0), stop=(k == KT - 1))
                if k == KT - 1:
                    nc.vector.tensor_copy(
                        v_aug[:, r, :, 0:HD],
                        st['ps'].rearrange("p (h d) -> p h d", d=HD))
            return [lambda k=k: step(k) for k in range(KT)]

        def weave(a, b, lead=1):
            """Alternate two step lists (a leads by `lead` steps) so each
            chain's accumulates are separated by the other chain's."""
            a, b = list(a), list(b)
            for j in range(lead):
                if a:
                    a.pop(0)()
            while a or b:
                if b:
                    b.pop(0)()
                if a:
                    a.pop(0)()

        def emit_qproj(c, p):
            for th in qproj_stepper(c, p):
                th()

        def emit_kproj_pair(p, cc0, cc1):
            weave(kproj_stepper(p, cc0), kproj_stepper(p, cc1))

        def emit_vproj_pair(r0, r1):
            weave(vproj_stepper(r0), vproj_stepper(r1))

        exp_map = {}   # (c, p) -> list of expP tiles

        def emit_S_pairs(c, p, ms):
            qA = qc_map[(c, p)][0:HD, :]
            qB = qc_map[(c, p)][HD:P, :]
            kTl = kT[p]
            lst = exp_map.setdefault((c, p), [None] * MT)
            for m in ms:
                if lst[m] is not None:
                    continue
                msl = slice(m * P, (m + 1) * P)
                s_ps = sps_pool.tile([P, 2 * CH], F32, tag="sps",
                                     name=f"sps_{c}_{p}_{m}")
                nc.tensor.matmul(s_ps[:, 0:CH], kTl[0:HD, msl], qA,
                                 start=True, stop=True)
                nc.tensor.matmul(s_ps[:, CH:2 * CH], kTl[HD:P, msl], qB,
                                 start=True, stop=True)
                expP = exp_pool.tile([P, 2 * CH], F16, tag="expp",
                                     name=f"expP_{c}_{p}_{m}")
                nc.scalar.activation(expP[:], s_ps[:],
                                     mybir.ActivationFunctionType.Exp,
                                     scale=SCALE)
                lst[m] = expP

        def emit_PV(c, p, fill, fillB=None, split_evac=False):
            # fill: dict slot -> list of thunks emitted before that PV matmul.
            # The two heads' PV accumulations run as separate contiguous
            # sweeps (A then B) so each group's weight loads pull ahead into
            # the background buffer instead of serializing on the group
            # switch; the A sweep is exp-paced, the B sweep runs dense.
            # split_evac (last unit): evacuate/broadcast the A head between
            # the sweeps so that DVE work overlaps the B sweep instead of
            # extending the kernel tail.
            oaugA = oaug_pool.tile([P, CH], F32, tag="oaug",
                                   name=f"oaugA_{c}_{p}")
            oaugB = oaug_pool.tile([P, CH], F32, tag="oaug",
                                   name=f"oaugB_{c}_{p}")
            expPs = exp_map.pop((c, p))
            for m in range(MT):
                for th in fill.get(m, ()):
                    th()
                vbase = (m * NH + 2 * p) * VW
                nc.tensor.matmul(oaugA[:, :], va_t[:, vbase:vbase + P],
                                 expPs[m][:, 0:CH],
                                 start=(m == 0), stop=(m == MT - 1))
            o_sb = osb_pool.tile([P, CH], F32, tag="osb", name=f"osb_{c}_{p}")
            bc = None
            if split_evac:
                denA = den_pool.tile([1, CH], F16, tag="den",
                                     name=f"denA_{c}_{p}")
                denB = den_pool.tile([1, CH], F16, tag="den",
                                     name=f"denB_{c}_{p}")
                nc.vector.tensor_copy(o_sb[0:HD, :], oaugA[0:HD, :])
                with nc.allow_low_precision(reason="softmax denom fp16"):
                    nc.vector.tensor_copy(denA[:], oaugA[HD:HD + 1, :])
                bc = aux_pool.tile([P, CH], F32, tag="aux", name=f"bc_{c}_{p}")
                nc.tensor.matmul(bc[0:HD, :], cst[0:1, 1:65], denA[:],
                                 start=True, stop=True)
            else:
                # both denominators in one tile (rows 0 and 32 — DVE writes
                # must start on a 32-aligned partition) so one matmul
                # broadcasts them to the two 64-partition halves at once.
                # Rows 1..31 must be zeroed: the broadcast matmul reads all
                # 33 partitions and 0 x garbage-NaN would poison the sum.
                den2 = den_pool.tile([33, CH], F16, tag="den",
                                     name=f"den2_{c}_{p}")
                nc.vector.memset(den2[0:32, :], 0.0)
            for m in range(MT):
                if fillB:
                    for th in fillB.get(m, ()):
                        th()
                vbase = (m * NH + 2 * p + 1) * VW
                nc.tensor.matmul(oaugB[:, :], va_t[:, vbase:vbase + P],
                                 expPs[m][:, CH:2 * CH],
                                 start=(m == 0), stop=(m == MT - 1))
            # evacuate denominators first (they head the norm critical
            # path: bc matmul -> reciprocal -> multiply), numerators after
            with nc.allow_low_precision(reason="softmax denom fp16"):
                if not split_evac:
                    nc.vector.tensor_copy(den2[0:1, :], oaugA[HD:HD + 1, :])
                    nc.vector.tensor_copy(den2[32:33, :], oaugB[HD:HD + 1, :])
                else:
                    nc.vector.tensor_copy(denB[:], oaugB[HD:HD + 1, :])
            if not split_evac:
                nc.vector.tensor_copy(o_sb[0:HD, :], oaugA[0:HD, :])
            nc.vector.tensor_copy(o_sb[HD:P, :], oaugB[0:HD, :])
            if not split_evac:
                return (c, p, o_sb, den2, None, None)
            return (c, p, o_sb, denA, denB, bc)

        ot_map = {}

        def emit_norm(unit):
            c, p, o_sb, dA, dB, bc = unit
            if bc is None:
                # dA is the merged [33, CH] denominator tile: one matmul
                # broadcasts denA (row 0) to partitions 0:64 and denB
                # (row 32) to 64:128; cst rows 1..31 are zero so the
                # garbage rows of dA contribute nothing
                bc = aux_pool.tile([P, CH], F32, tag="aux",
                                   name=f"bc_{c}_{p}")
                nc.tensor.matmul(bc[:], cst[0:33, 1:129], dA[:],
                                 start=True, stop=True)
            else:
                nc.tensor.matmul(bc[HD:P, :], cst[0:1, 1:65], dB[:],
                                 start=True, stop=True)
            rbc = rbc_pool.tile([P, CH], F32, tag="rbc", name=f"rbc_{c}_{p}")
            nc.vector.reciprocal_approx_fast(out=rbc[:], in_=bc[:])
            ot_p = ot_pool.tile([P, CH], F16, name=f"ot_{c}_{p}", tag="ot")
            nc.vector.tensor_tensor(ot_p[:], o_sb[:], rbc[:],
                                    mybir.AluOpType.mult)
            ot_map[(c, p)] = ot_p

        # ---- the pipeline -----------------------------------------------
        units = [(c, p) for c in range(NCHUNK) for p in range(NPAIR)]

        # out-projection drain: a stepper that emits one pair-contribution
        # matmul per call, round-robining over up to two open e-block
        # chains so consecutive accumulates never hit the same PSUM tile
        pend_outproj = deque()   # (chunk, e) blocks ready to drain
        op_active = []

        def outproj_step(n=1):
            for _ in range(n):
                while len(op_active) < max(n, 1) and pend_outproj:
                    c2, e = pend_outproj.popleft()
                    op_active.append({'c': c2, 'e': e, 'pp': 0})
                if not op_active:
                    return
                stt = op_active.pop(0)
                c2, e, pp = stt['c'], stt['e'], stt['pp']
                if pp == 0:
                    stt['ps'] = aux_pool.tile([P, CH], F32, tag="aux",
                                              name=f"pso_{c2}_{e}")
                nc.tensor.matmul(stt['ps'][:], wo[pp][:, e * P:(e + 1) * P],
                                 ot_map[(c2, pp)][:],
                                 start=(pp == 0), stop=(pp == NPAIR - 1))
                stt['pp'] += 1
                if stt['pp'] == NPAIR:
                    stg = stage_pool.tile([P, CH], F16, tag="stg",
                                          name=f"st_{c2}_{e}")
                    with nc.allow_low_precision(reason="fp16 output partials"):
                        nc.vector.tensor_copy(stg[:], stt['ps'][:])
                    nc.sync.dma_start(
                        outT_r[e][:, slice(c2 * CH, (c2 + 1) * CH)], stg[:])
                else:
                    op_active.append(stt)

        # prologue: unit (0,0) S phase with k-projection per chunk and the
        # first half of the v-projection woven in. This region is paced by
        # the cold input DMA stream, so the chains run in strict data-
        # arrival order (the PSUM accumulate hazards are hidden behind the
        # transfer waits anyway).
        def run_chain(steps):
            for th in steps:
                th()

        for cc in range(NCHUNK):
            run_chain(kproj_stepper(0, cc))
            if cc == 0:
                emit_qproj(0, 0)
            emit_S_pairs(0, 0, range(4 * cc, 4 * cc + 4))
            if cc < 2:
                for r in range(4 * cc, 4 * cc + 4):
                    run_chain(vproj_stepper(r))
        # projections for pair 1 / the (0,1) and (1,0) q-chunks; their
        # hoisted S pairs are NOT emitted here — they go into unit (0,0)'s
        # fill slots below, so PV-A(0,0) isn't queued behind an ACT-paced
        # block of S pairs stalling on S-PSUM recycling
        for cc in range(NCHUNK):
            run_chain(kproj_stepper(1, cc))
        emit_qproj(0, 1)
        emit_qproj(1, 0)
        vsteps7 = []

        pend_norm = deque()
        normed = {c: 0 for c in range(NCHUNK)}
        op3_tiles = []
        op3_slices = {}

        for i, (c, p) in enumerate(units):
            nxt = units[i + 1] if i + 1 < len(units) else None
            nxt2 = units[i + 2] if i + 2 < len(units) else None

            fill = {}
            fillB = {}
            if nxt is not None:
                # second half of the next unit's S phase at the even slots
                for j, m in enumerate(range(8, MT)):
                    fill.setdefault(j * 2, []).append(
                        lambda u=nxt, mm=m: emit_S_pairs(u[0], u[1], [mm]))
            if i < 2:
                # chunk-0 special units: k-projections for the later pairs
                # as chunk-pair-interleaved chains, v-projection row-blocks
                # just-in-time for PV, monolithic q-projection — layout as
                # in the original schedule
                if (c, p) == (0, 0):
                    for j, th in enumerate(vsteps7):
                        fill.setdefault(j, []).append(th)
                    for j, r in enumerate(range(8, MT, 2)):
                        fill.setdefault(8 + 2 * j, []).append(
                            lambda r0=r: emit_vproj_pair(r0, r0 + 1))
                    # the exp hoist for units (0,1) and (1,0), spread over
                    # this unit's free slots: banks two units of exps while
                    # PV and projection work keeps the PE busy between the
                    # ACT-paced S pairs
                    hoist = ([(0, 1, m) for m in range(0, 8)]
                             + [(1, 0, m) for m in range(0, 6)])
                    hslots = [(fill, 5), (fill, 7), (fillB, 0), (fillB, 1),
                              (fillB, 3), (fillB, 4), (fillB, 5), (fillB, 7),
                              (fillB, 8), (fillB, 9), (fillB, 11),
                              (fillB, 12), (fillB, 13), (fillB, 15)]
                    for (c2, p2, m), (dct, s) in zip(hoist, hslots):
                        dct.setdefault(s, []).append(
                            lambda a=c2, b=p2, mm=m: emit_S_pairs(a, b, [mm]))
                if nxt2 is not None:
                    p2 = nxt2[1]
                    fill.setdefault(1, []).append(
                        lambda p2=p2: emit_kproj_pair(p2, 0, 1))
                    fill.setdefault(3, []).append(
                        lambda p2=p2: emit_kproj_pair(p2, 2, 3))
                    fill.setdefault(8, []).append(
                        lambda c2=nxt2[0], p2=p2: emit_qproj(c2, p2))
                    for j, m in enumerate(range(0, 4)):
                        fill.setdefault(9 + 2 * j, []).append(
                            lambda u=nxt2, mm=m: emit_S_pairs(u[0], u[1],
                                                              [mm]))
                    for j, m in enumerate(range(4, 8)):
                        fillB.setdefault(2 + 4 * j, []).append(
                            lambda u=nxt2, mm=m: emit_S_pairs(u[0], u[1],
                                                              [mm]))
            else:
                # steady state: every PV accumulate gap carries >=213ns of
                # PE work so the ~210ns same-tile PSUM hazard stays hidden.
                # PV-A odds: q-projection of the unit-after-next, per k.
                # PV-B: S-firsts of the unit-after-next on slots 2,6,10,14
                # (m 0..3) and 1,5,9,13 (m 4..7); out-projection drain
                # steps on the remaining 8 slots.
                if nxt2 is not None:
                    qsteps = qproj_stepper(nxt2[0], nxt2[1])
                    for j, thq in enumerate(qsteps):
                        fill.setdefault(2 * j + 1, []).append(thq)
                    for j, m in enumerate(range(0, 4)):
                        fillB.setdefault(2 + 4 * j, []).append(
                            lambda u=nxt2, mm=m: emit_S_pairs(u[0], u[1],
                                                              [mm]))
                    for j, m in enumerate(range(4, 8)):
                        fillB.setdefault(1 + 4 * j, []).append(
                            lambda u=nxt2, mm=m: emit_S_pairs(u[0], u[1],
                                                              [mm]))
                if nxt2 is None:
                    # final two units have no q-proj/S-firsts to place —
                    # keep the PV-A gaps fed with out-projection drain
                    # steps instead so the PE never idles into the tail
                    for s in (1, 3, 5, 7, 9, 11, 13, 15):
                        fill.setdefault(s, []).append(
                            lambda: outproj_step(1))
                for s in (0, 3, 4, 7, 8, 11, 12, 15):
                    fillB.setdefault(s, []).append(
                        lambda: outproj_step(1))
            if (c, p) == (NCHUNK - 1, NPAIR - 1):
                # pre-accumulate the last chunk's out-projection over pairs
                # 0..2 for e=0..3 while this unit's PV finishes; pair 3 is
                # added after its norm in the tail
                def partial(e):
                    if e % 2 == 0:
                        op3_tiles.append(
                            sps_pool.tile([P, 2 * CH], F32, tag="sps",
                                          name=f"op3_{e}"))
                    half = op3_tiles[e // 2][:, (e % 2) * CH:(e % 2 + 1) * CH]
                    op3_slices[e] = half
                    for pp in range(3):
                        nc.tensor.matmul(
                            half, wo[pp][:, e * P:(e + 1) * P],
                            ot_map[(NCHUNK - 1, pp)][:],
                            start=(pp == 0), stop=False)
                for j in range(4):
                    fillB.setdefault(2 + 4 * j, []).append(
                        lambda e=j: partial(e))
                # the partials need ot of pairs 0..2 — drain the norm queue
                while pend_norm:
                    u = pend_norm.popleft()
                    emit_norm(u)
                    normed[u[0]] += 1

            unit = emit_PV(c, p, fill, fillB,
                           split_evac=((c, p) == (NCHUNK - 1, NPAIR - 1)))
            pend_norm.append(unit)
            # lagged norm (immediate after the last unit)
            while pend_norm and (len(pend_norm) > 1
                                 or (c, p) == (NCHUNK - 1, NPAIR - 1)):
                u = pend_norm.popleft()
                emit_norm(u)
                normed[u[0]] += 1
                if normed[u[0]] == NPAIR and u[0] < NCHUNK - 1:
                    cc = u[0]
                    for e in range(ET):
                        pend_outproj.append((cc, e))

        # tail: any leftover drain blocks, then finish the pre-accumulated
        # e-blocks (pair 3), then the remaining e-blocks pairwise
        # interleaved; stores fan out over all three DMA rings (the scalar
        # ring is free again — the last exp is long gone)
        while pend_outproj or op_active:
            outproj_step(2)
        # the last chunk's ~1MB of stores move as 1KB packets — far below
        # ring burst rate — so fan them out over all three DMA rings (the
        # scalar ring is free again, the last exp is long gone)
        rings = [nc.sync, nc.scalar, nc.gpsimd]
        csl3 = slice((NCHUNK - 1) * CH, NCHUNK * CH)
        for e in range(4):
            half = op3_slices[e]
            nc.tensor.matmul(half, wo[3][:, e * P:(e + 1) * P],
                             ot_map[(NCHUNK - 1, 3)][:],
                             start=False, stop=True)
            st = stage_pool.tile([P, CH], F16, tag="stg", name=f"st3_{e}")
            with nc.allow_low_precision(reason="fp16 output partials"):
                nc.vector.tensor_copy(st[:], half)
            rings[e % 3].dma_start(outT_r[e][:, csl3], st[:])
        for e0 in (4, 6):
            # the (3,3) oaug accumulators are already evacuated here, so
            # their pool is free — using it keeps these chains off the
            # aux pool, whose buffers still WAR-wait on the last drain
            # blocks' casts
            psA = oaug_pool.tile([P, CH], F32, tag="oaug", name=f"pso3_{e0}")
            psB = oaug_pool.tile([P, CH], F32, tag="oaug",
                                 name=f"pso3_{e0 + 1}")
            for pp in range(NPAIR):
                nc.tensor.matmul(psA[:], wo[pp][:, e0 * P:(e0 + 1) * P],
                                 ot_map[(NCHUNK - 1, pp)][:],
                                 start=(pp == 0), stop=(pp == NPAIR - 1))
                nc.tensor.matmul(psB[:],
                                 wo[pp][:, (e0 + 1) * P:(e0 + 2) * P],
                                 ot_map[(NCHUNK - 1, pp)][:],
                                 start=(pp == 0), stop=(pp == NPAIR - 1))
            for j, ps in ((0, psA), (1, psB)):
                e = e0 + j
                st = stage_pool.tile([P, CH], F16, tag="stg",
                                     name=f"st3_{e}")
                with nc.allow_low_precision(reason="fp16 output partials"):
                    nc.vector.tensor_copy(st[:], ps[:])
                rings[e % 3].dma_start(outT_r[e][:, csl3], st[:])

    nc.compile()
    return nc


def _get_nc():
    global _NC_CACHE
    if _NC_CACHE is None:
        _NC_CACHE = _build()
    return _NC_CACHE


def _make_in_maps(x, w_qkv, w_out):
    cst = np.zeros((P, 129), dtype=np.float16)
    cst[:, 0] = 1.0
    cst[0, 1:65] = 1.0
    cst[32, 65:129] = 1.0
    per_g = []
    for g in range(2):
        qk_g = np.concatenate([w_qkv[g * 512:(g + 1) * 512],
                               w_qkv[DI + g * 512:DI + (g + 1) * 512]], axis=0)
        wqkT = np.ascontiguousarray(qk_g.T)               # [1024 d, 1024 f]
        wqkA = np.ascontiguousarray(
            wqkT.reshape(KT, P, 8, P).transpose(2, 1, 0, 3).astype(np.float16))
        v_g = w_qkv[2 * DI + g * 512:2 * DI + (g + 1) * 512]
        wvT = np.ascontiguousarray(v_g.T)                 # [1024 d, 512 f]
        wvA = np.ascontiguousarray(
            wvT.reshape(KT, P, 512).transpose(1, 0, 2).astype(np.float16))
        woTg = np.ascontiguousarray(
            w_out[:, g * 512:(g + 1) * 512].T.astype(np.float16))
        per_g.append((wqkA, wvA, woTg))

    in_maps = []
    for c in range(8):
        b, g = c // 2, c % 2
        wqkA, wvA, woTg = per_g[g]
        in_maps.append({
            "xT": np.ascontiguousarray(x[b].T.astype(np.float16)),
            "wqkA": wqkA,
            "wvA": wvA,
            "woT": woTg,
            "cst": cst,
        })
    return in_maps


def kernel(x, w_qkv, w_out, b_out):
    x = np.asarray(x, dtype=np.float32)
    w_qkv = np.asarray(w_qkv, dtype=np.float32)
    w_out = np.asarray(w_out, dtype=np.float32)
    b_out = np.asarray(b_out, dtype=np.float32)
    B = x.shape[0]

    in_maps = _make_in_maps(x, w_qkv, w_out)
    nc = _get_nc()
    res = run_bass_kernel_spmd(nc, in_maps, core_ids=list(range(8)))
    parts = [r["outT"] for r in res.results]
    out = np.empty((B, N, DI), dtype=np.float32)
    for b in range(B):
        out[b] = (parts[2 * b].astype(np.float32)
                  + parts[2 * b + 1].astype(np.float32)).T + b_out
    return out



# revision 44
# speedup vs baseline: 1.0093x; 1.0093x over previous
"""Multi-head attention Trainium2 kernel (B=4, N=2048, D=1024, H=16).

Sharding: 8 cores = 4 batches x 2 head-groups (8 heads each), zero
collectives. Each core:
  - all projections in fp16, interleaved into the attention pipeline so
    the scalar engine (exp) starts ~10us in and stays saturated:
    x arrives as 512-column slices; the first head-pair's k-projection and
    q-projection run as the slices land, its S matmuls follow immediately,
    and the v-projection row-blocks are emitted just-in-time before the
    PV matmul that consumes them
  - q,k kept transposed [feat, seq]; v row-layout, augmented with a ones
    column so the PV matmul emits the softmax denominator for free
  - attention per head-pair x 512-query chunk: S matmuls packed two heads
    per pass via disjoint PE row groups into one [128,1024] PSUM tile,
    one wide exp on ACT (scale=1/8, fp16 out), PV accumulation with
    128-column stationary windows (fast-weight-load path)
  - software pipeline: each unit's S phase is split 8/8 around the
    previous unit's PV loop (exp pool holds up to 34 tiles) so the scalar
    engine never starves; projection/out-projection/normalization work
    fills the PE slack inside the ACT-paced PV loops; the last chunk's
    out-projection is pre-accumulated over pairs 0..2 during the final
    unit and finished right after its norm
  - normalization: one 64-wide ones matmul per head broadcasts the fp16
    denominator across partitions, one reciprocal_approx_fast and one
    multiply produce the normalized [128, 512] fp16 tile per unit
  - out-projection partial [1024,2048] per chunk, evacuated via DVE
Host sums the two head-group partials per batch and adds bias.
"""
from collections import deque
from contextlib import ExitStack

import numpy as np

import concourse.mybir as mybir
import concourse.tile as tile
from concourse import bacc
from concourse.bass_utils import run_bass_kernel_spmd

F32 = mybir.dt.float32
F16 = mybir.dt.float16

P = 128
N = 2048         # sequence length
DI = 1024        # model dim
NH = 8           # heads per core
HD = 64          # head dim
NPAIR = 4        # head pairs per core
KT = 8           # contraction tiles for projections
CH = 512         # query chunk width
NCHUNK = 4       # chunks per sequence
MT = 16          # key tiles (m) per sequence
ET = 8           # output-feature blocks
SCALE = HD ** -0.5
VW = HD + 1      # v columns per head incl. denominator ones-column
VFLAT = MT * NH * VW

_NC_CACHE = None


def _build():
    nc = bacc.Bacc("TRN2", target_bir_lowering=False, debug=False)

    xT = nc.dram_tensor("xT", [DI, N], F16, kind="ExternalInput").ap()
    wqkA = nc.dram_tensor("wqkA", [8, P, KT, P], F16, kind="ExternalInput").ap()
    wvA = nc.dram_tensor("wvA", [P, KT, 512], F16, kind="ExternalInput").ap()
    woT = nc.dram_tensor("woT", [512, DI], F16, kind="ExternalInput").ap()
    cstd = nc.dram_tensor("cst", [P, 129], F16, kind="ExternalInput").ap()
    outT = nc.dram_tensor("outT", [DI, N], F16, kind="ExternalOutput").ap()

    xT_r = xT.rearrange("(k p) n -> k p n", p=P)        # [8, 128, 2048]
    woT_r = woT.rearrange("(k p) e -> k p e", p=P)      # [4, 128, 1024]
    outT_r = outT.rearrange("(e p) n -> e p n", p=P)    # [8, 128, 2048]

    with tile.TileContext(nc) as tc, ExitStack() as persist:
        qk_pool = persist.enter_context(tc.tile_pool(name="qkp", bufs=4))
        va_pool = persist.enter_context(tc.tile_pool(name="vap", bufs=1))
        misc = persist.enter_context(tc.tile_pool(name="misc", bufs=1))
        wqk_pool = persist.enter_context(tc.tile_pool(name="wqk", bufs=2))
        xt_pool = persist.enter_context(tc.tile_pool(name="xt", bufs=8))
        wv_pool = persist.enter_context(tc.tile_pool(name="wv", bufs=1))
        wq_pool = persist.enter_context(tc.tile_pool(name="wq", bufs=4))
        wo_pool = persist.enter_context(tc.tile_pool(name="wo", bufs=4))
        exp_pool = persist.enter_context(tc.tile_pool(name="expp", bufs=36))
        ot_pool = persist.enter_context(tc.tile_pool(name="ot", bufs=8))
        osb_pool = persist.enter_context(tc.tile_pool(name="osb", bufs=4))
        stage_pool = persist.enter_context(tc.tile_pool(name="stg", bufs=6))
        den_pool = persist.enter_context(tc.tile_pool(name="den", bufs=8))
        rbc_pool = persist.enter_context(tc.tile_pool(name="rbc", bufs=2))
        sps_pool = persist.enter_context(
            tc.tile_pool(name="sps", bufs=2, space="PSUM"))
        oaug_pool = persist.enter_context(
            tc.tile_pool(name="oaug", bufs=2, space="PSUM"))
        aux_pool = persist.enter_context(
            tc.tile_pool(name="aux", bufs=2, space="PSUM"))

        # --- input DMAs, ordered so the pipeline lights up ASAP: the
        # first k-projection group needs wqk[4] + the chunk-0 columns of
        # every x k-tile; q needs wq0; v needs wv. Early loads are split
        # across the two HWDGE rings (sync + scalar) so the startup
        # transfers run in parallel; the scalar ring is only used before
        # the first exp is queued, so the triggers never stall ACT.
        wqk_first = wqk_pool.tile([P, KT, P], F16, tag="wqk")
        nc.sync.dma_start(wqk_first[:], wqkA[4])
        cst = misc.tile([P, 129], F16)
        xt = [xt_pool.tile([P, N], F16, name=f"xt{k}", tag="xt")
              for k in range(KT)]
        wq = [wq_pool.tile([P, KT, P], F16, name=f"wq{f}", tag="wq")
              for f in range(4)]
        # x arrives in 1024-column halves: per-partition lines are 2KB
        # instead of 1KB, halving the packet count for the same data — the
        # cold DMA phase is packet-count-bound, so chunks 0+1 land in the
        # time chunk 0 alone used to take
        for k in range(KT):
            nc.sync.dma_start(xt[k][:, 0:2 * CH], xT_r[k][:, 0:2 * CH])
        nc.sync.dma_start(wq[0][:], wqkA[0])
        nc.sync.dma_start(cst[:], cstd[:])
        wv = wv_pool.tile([P, KT, 512], F16)
        nc.sync.dma_start(wv[:], wvA[:])
        wqk_tiles = {4: wqk_first}
        nc.sync.dma_start(wq[1][:], wqkA[1])
        for k in range(KT):
            nc.sync.dma_start(xt[k][:, 2 * CH:N], xT_r[k][:, 2 * CH:N])
        nc.sync.dma_start(wq[2][:], wqkA[2])
        nc.sync.dma_start(wq[3][:], wqkA[3])
        wo = [wo_pool.tile([P, DI], F16, name=f"wo{kk}", tag="wo")
              for kk in range(NPAIR)]
        for kk in range(NPAIR):
            nc.sync.dma_start(wo[kk][:], woT_r[kk])

        # k tiles persist (full sequence, read by every chunk); q chunks
        # rotate through a small pool — each q(c,p) is dead once unit (c,p)'s
        # S phase completes, so 6 buffers cover the pipeline depth
        kT = [qk_pool.tile([P, N], F16, name=f"kT{t}", tag="qkT")
              for t in range(4)]
        qc_pool = persist.enter_context(tc.tile_pool(name="qc", bufs=6))
        qc_map = {}
        va_t = va_pool.tile([P, VFLAT + 64], F16)
        nc.vector.memset(va_t[:, VFLAT:VFLAT + 64], 0.0)
        v_aug = va_t[:, 0:VFLAT].rearrange("p (m h d) -> p m h d", h=NH, d=VW)
        nc.vector.tensor_copy(v_aug[:, :, :, HD:HD + 1],
                              cst[:, 0:1].to_broadcast((P, MT, NH, 1)))

        # ---- emission helpers -------------------------------------------
        # Projection chains accumulate 8 k-tiles into one PSUM tile.
        # Back-to-back accumulates into the same PSUM region pay a ~210ns
        # RAW hazard (the previous matmul's PSUM writes must retire), so
        # every chain is emitted as per-k "steps" that get woven between
        # other matmuls — either another chain (weave/pairs) or the PV
        # matmuls of the surrounding sweep (fill slots).
        kdone = set()

        def kproj_stepper(p, cc):
            if (p, cc) in kdone:
                return []
            kdone.add((p, cc))
            f = 4 + p
            if f not in wqk_tiles:
                t = wqk_pool.tile([P, KT, P], F16, tag="wqk")
                nc.sync.dma_start(t[:], wqkA[f])
                wqk_tiles[f] = t
            st = {}
            csl = slice(cc * CH, (cc + 1) * CH)

            def step(k):
                if 'ps' not in st:
                    st['ps'] = aux_pool.tile([P, CH], F32, tag="aux",
                                             name=f"kp_{p}_{cc}")
                nc.tensor.matmul(st['ps'][:], wqk_tiles[f][:, k, :],
                                 xt[k][:, csl],
                                 start=(k == 0), stop=(k == KT - 1))
                if k == KT - 1:
                    nc.vector.tensor_copy(kT[p][:, csl], st['ps'][:])
            return [lambda k=k: step(k) for k in range(KT)]

        qp_started = set()

        def qproj_stepper(c, p):
            if (c, p) in qc_map or (c, p) in qp_started:
                return []
            qp_started.add((c, p))
            st = {}
            csl = slice(c * CH, (c + 1) * CH)

            def step(k):
                if 'ps' not in st:
                    st['ps'] = aux_pool.tile([P, CH], F32, tag="aux",
                                             name=f"qp_{c}_{p}")
                nc.tensor.matmul(st['ps'][:], wq[p][:, k, :], xt[k][:, csl],
                                 start=(k == 0), stop=(k == KT - 1))
                if k == KT - 1:
                    qc = qc_pool.tile([P, CH], F16, name=f"qc_{c}_{p}",
                                      tag="qc")
                    nc.vector.tensor_copy(qc[:], st['ps'][:])
                    qc_map[(c, p)] = qc
            return [lambda k=k: step(k) for k in range(KT)]

        def vproj_stepper(r):
            st = {}

            def step(k):
                if 'ps' not in st:
                    st['ps'] = aux_pool.tile([P, CH], F32, tag="aux",
                                             name=f"vp_{r}")
                nc.tensor.matmul(st['ps'][:], xt[k][:, r * P:(r + 1) * P],
                                 wv[:, k, :],
                                 start=(k == 0), stop=(k == KT - 1))
                if k == KT - 1:
                    nc.vector.tensor_copy(
                        v_aug[:, r, :, 0:HD],
                        st['ps'].rearrange("p (h d) -> p h d", d=HD))
            return [lambda k=k: step(k) for k in range(KT)]

        def weave(a, b, lead=1):
            """Alternate two step lists (a leads by `lead` steps) so each
            chain's accumulates are separated by the other chain's."""
            a, b = list(a), list(b)
            for j in range(lead):
                if a:
                    a.pop(0)()
            while a or b:
                if b:
                    b.pop(0)()
                if a:
                    a.pop(0)()

        def emit_qproj(c, p):
            for th in qproj_stepper(c, p):
                th()

        def emit_kproj_pair(p, cc0, cc1):
            weave(kproj_stepper(p, cc0), kproj_stepper(p, cc1))

        def emit_vproj_pair(r0, r1):
            weave(vproj_stepper(r0), vproj_stepper(r1))

        exp_map = {}   # (c, p) -> list of expP tiles

        def emit_S_pairs(c, p, ms):
            qA = qc_map[(c, p)][0:HD, :]
            qB = qc_map[(c, p)][HD:P, :]
            kTl = kT[p]
            lst = exp_map.setdefault((c, p), [None] * MT)
            for m in ms:
                if lst[m] is not None:
                    continue
                msl = slice(m * P, (m + 1) * P)
                s_ps = sps_pool.tile([P, 2 * CH], F32, tag="sps",
                                     name=f"sps_{c}_{p}_{m}")
                nc.tensor.matmul(s_ps[:, 0:CH], kTl[0:HD, msl], qA,
                                 start=True, stop=True)
                nc.tensor.matmul(s_ps[:, CH:2 * CH], kTl[HD:P, msl], qB,
                                 start=True, stop=True)
                expP = exp_pool.tile([P, 2 * CH], F16, tag="expp",
                                     name=f"expP_{c}_{p}_{m}")
                nc.scalar.activation(expP[:], s_ps[:],
                                     mybir.ActivationFunctionType.Exp,
                                     scale=SCALE)
                lst[m] = expP

        def emit_PV(c, p, fill, fillB=None, split_evac=False):
            # fill: dict slot -> list of thunks emitted before that PV matmul.
            # The two heads' PV accumulations run as separate contiguous
            # sweeps (A then B) so each group's weight loads pull ahead into
            # the background buffer instead of serializing on the group
            # switch; the A sweep is exp-paced, the B sweep runs dense.
            # split_evac (last unit): evacuate/broadcast the A head between
            # the sweeps so that DVE work overlaps the B sweep instead of
            # extending the kernel tail.
            oaugA = oaug_pool.tile([P, CH], F32, tag="oaug",
                                   name=f"oaugA_{c}_{p}")
            oaugB = oaug_pool.tile([P, CH], F32, tag="oaug",
                                   name=f"oaugB_{c}_{p}")
            expPs = exp_map.pop((c, p))
            for m in range(MT):
                for th in fill.get(m, ()):
                    th()
                vbase = (m * NH + 2 * p) * VW
                nc.tensor.matmul(oaugA[:, :], va_t[:, vbase:vbase + P],
                                 expPs[m][:, 0:CH],
                                 start=(m == 0), stop=(m == MT - 1))
            o_sb = osb_pool.tile([P, CH], F32, tag="osb", name=f"osb_{c}_{p}")
            bc = None
            if split_evac:
                denA = den_pool.tile([1, CH], F16, tag="den",
                                     name=f"denA_{c}_{p}")
                denB = den_pool.tile([1, CH], F16, tag="den",
                                     name=f"denB_{c}_{p}")
                nc.vector.tensor_copy(o_sb[0:HD, :], oaugA[0:HD, :])
                with nc.allow_low_precision(reason="softmax denom fp16"):
                    nc.vector.tensor_copy(denA[:], oaugA[HD:HD + 1, :])
                bc = aux_pool.tile([P, CH], F32, tag="aux", name=f"bc_{c}_{p}")
                nc.tensor.matmul(bc[0:HD, :], cst[0:1, 1:65], denA[:],
                                 start=True, stop=True)
            else:
                # both denominators in one tile (rows 0 and 32 — DVE writes
                # must start on a 32-aligned partition) so one matmul
                # broadcasts them to the two 64-partition halves at once.
                # Rows 1..31 must be zeroed: the broadcast matmul reads all
                # 33 partitions and 0 x garbage-NaN would poison the sum.
                den2 = den_pool.tile([33, CH], F16, tag="den",
                                     name=f"den2_{c}_{p}")
                nc.vector.memset(den2[0:32, :], 0.0)
            for m in range(MT):
                if fillB:
                    for th in fillB.get(m, ()):
                        th()
                vbase = (m * NH + 2 * p + 1) * VW
                nc.tensor.matmul(oaugB[:, :], va_t[:, vbase:vbase + P],
                                 expPs[m][:, CH:2 * CH],
                                 start=(m == 0), stop=(m == MT - 1))
            # evacuate denominators first (they head the norm critical
            # path: bc matmul -> reciprocal -> multiply), numerators after
            with nc.allow_low_precision(reason="softmax denom fp16"):
                if not split_evac:
                    nc.vector.tensor_copy(den2[0:1, :], oaugA[HD:HD + 1, :])
                    nc.vector.tensor_copy(den2[32:33, :], oaugB[HD:HD + 1, :])
                else:
                    nc.vector.tensor_copy(denB[:], oaugB[HD:HD + 1, :])
            if not split_evac:
                nc.vector.tensor_copy(o_sb[0:HD, :], oaugA[0:HD, :])
            nc.vector.tensor_copy(o_sb[HD:P, :], oaugB[0:HD, :])
            if not split_evac:
                return (c, p, o_sb, den2, None, None)
            return (c, p, o_sb, denA, denB, bc)

        ot_map = {}

        def emit_norm(unit):
            c, p, o_sb, dA, dB, bc = unit
            if bc is None:
                # dA is the merged [33, CH] denominator tile: one matmul
                # broadcasts denA (row 0) to partitions 0:64 and denB
                # (row 32) to 64:128; cst rows 1..31 are zero so the
                # garbage rows of dA contribute nothing
                bc = aux_pool.tile([P, CH], F32, tag="aux",
                                   name=f"bc_{c}_{p}")
                nc.tensor.matmul(bc[:], cst[0:33, 1:129], dA[:],
                                 start=True, stop=True)
            else:
                nc.tensor.matmul(bc[HD:P, :], cst[0:1, 1:65], dB[:],
                                 start=True, stop=True)
            rbc = rbc_pool.tile([P, CH], F32, tag="rbc", name=f"rbc_{c}_{p}")
            nc.vector.reciprocal_approx_fast(out=rbc[:], in_=bc[:])
            ot_p = ot_pool.tile([P, CH], F16, name=f"ot_{c}_{p}", tag="ot")
            nc.vector.tensor_tensor(ot_p[:], o_sb[:], rbc[:],
                                    mybir.AluOpType.mult)
            ot_map[(c, p)] = ot_p

        # ---- the pipeline -----------------------------------------------
        units = [(c, p) for c in range(NCHUNK) for p in range(NPAIR)]

        # out-projection drain: a stepper that emits one pair-contribution
        # matmul per call, round-robining over up to two open e-block
        # chains so consecutive accumulates never hit the same PSUM tile
        pend_outproj = deque()   # (chunk, e) blocks ready to drain
        op_active = []

        def outproj_step(n=1):
            for _ in range(n):
                while len(op_active) < max(n, 1) and pend_outproj:
                    c2, e = pend_outproj.popleft()
                    op_active.append({'c': c2, 'e': e, 'pp': 0})
                if not op_active:
                    return
                stt = op_active.pop(0)
                c2, e, pp = stt['c'], stt['e'], stt['pp']
                if pp == 0:
                    stt['ps'] = aux_pool.tile([P, CH], F32, tag="aux",
                                              name=f"pso_{c2}_{e}")
                nc.tensor.matmul(stt['ps'][:], wo[pp][:, e * P:(e + 1) * P],
                                 ot_map[(c2, pp)][:],
                                 start=(pp == 0), stop=(pp == NPAIR - 1))
                stt['pp'] += 1
                if stt['pp'] == NPAIR:
                    stg = stage_pool.tile([P, CH], F16, tag="stg",
                                          name=f"st_{c2}_{e}")
                    with nc.allow_low_precision(reason="fp16 output partials"):
                        nc.vector.tensor_copy(stg[:], stt['ps'][:])
                    nc.sync.dma_start(
                        outT_r[e][:, slice(c2 * CH, (c2 + 1) * CH)], stg[:])
                else:
                    op_active.append(stt)

        # prologue: unit (0,0) S phase with k-projection per chunk and the
        # first half of the v-projection woven in. This region is paced by
        # the cold input DMA stream, so the chains run in strict data-
        # arrival order (the PSUM accumulate hazards are hidden behind the
        # transfer waits anyway).
        def run_chain(steps):
            for th in steps:
                th()

        for cc in range(NCHUNK):
            run_chain(kproj_stepper(0, cc))
            if cc == 0:
                emit_qproj(0, 0)
            emit_S_pairs(0, 0, range(4 * cc, 4 * cc + 4))
            if cc < 2:
                for r in range(4 * cc, 4 * cc + 4):
                    run_chain(vproj_stepper(r))
        # projections for pair 1 / the (0,1) and (1,0) q-chunks; their
        # hoisted S pairs are NOT emitted here — they go into unit (0,0)'s
        # fill slots below, so PV-A(0,0) isn't queued behind an ACT-paced
        # block of S pairs stalling on S-PSUM recycling
        for cc in range(NCHUNK):
            run_chain(kproj_stepper(1, cc))
        emit_qproj(0, 1)
        emit_qproj(1, 0)
        vsteps7 = []

        pend_norm = deque()
        normed = {c: 0 for c in range(NCHUNK)}
        op3_tiles = []
        op3_slices = {}

        for i, (c, p) in enumerate(units):
            nxt = units[i + 1] if i + 1 < len(units) else None
            nxt2 = units[i + 2] if i + 2 < len(units) else None

            fill = {}
            fillB = {}
            if nxt is not None:
                # second half of the next unit's S phase at the even slots
                for j, m in enumerate(range(8, MT)):
                    fill.setdefault(j * 2, []).append(
                        lambda u=nxt, mm=m: emit_S_pairs(u[0], u[1], [mm]))
            if i < 2:
                # chunk-0 special units: k-projections for the later pairs
                # as chunk-pair-interleaved chains, v-projection row-blocks
                # just-in-time for PV, monolithic q-projection — layout as
                # in the original schedule
                if (c, p) == (0, 0):
                    for j, th in enumerate(vsteps7):
                        fill.setdefault(j, []).append(th)
                    for j, r in enumerate(range(8, MT, 2)):
                        fill.setdefault(8 + 2 * j, []).append(
                            lambda r0=r: emit_vproj_pair(r0, r0 + 1))
                    # the exp hoist for units (0,1) and (1,0), spread over
                    # this unit's free slots: banks two units of exps while
                    # PV and projection work keeps the PE busy between the
                    # ACT-paced S pairs
                    hoist = ([(0, 1, m) for m in range(0, 8)]
                             + [(1, 0, m) for m in range(0, 6)])
                    hslots = [(fill, 5), (fill, 7), (fillB, 0), (fillB, 1),
                              (fillB, 3), (fillB, 4), (fillB, 5), (fillB, 7),
                              (fillB, 8), (fillB, 9), (fillB, 11),
                              (fillB, 12), (fillB, 13), (fillB, 15)]
                    for (c2, p2, m), (dct, s) in zip(hoist, hslots):
                        dct.setdefault(s, []).append(
                            lambda a=c2, b=p2, mm=m: emit_S_pairs(a, b, [mm]))
                if nxt2 is not None:
                    p2 = nxt2[1]
                    fill.setdefault(1, []).append(
                        lambda p2=p2: emit_kproj_pair(p2, 0, 1))
                    fill.setdefault(3, []).append(
                        lambda p2=p2: emit_kproj_pair(p2, 2, 3))
                    fill.setdefault(8, []).append(
                        lambda c2=nxt2[0], p2=p2: emit_qproj(c2, p2))
                    for j, m in enumerate(range(0, 4)):
                        fill.setdefault(9 + 2 * j, []).append(
                            lambda u=nxt2, mm=m: emit_S_pairs(u[0], u[1],
                                                              [mm]))
                    for j, m in enumerate(range(4, 8)):
                        fillB.setdefault(2 + 4 * j, []).append(
                            lambda u=nxt2, mm=m: emit_S_pairs(u[0], u[1],
                                                              [mm]))
            else:
                # steady state: every PV accumulate gap carries >=213ns of
                # PE work so the ~210ns same-tile PSUM hazard stays hidden.
                # PV-A odds: q-projection of the unit-after-next, per k.
                # PV-B: S-firsts of the unit-after-next on slots 2,6,10,14
                # (m 0..3) and 1,5,9,13 (m 4..7); out-projection drain
                # steps on the remaining 8 slots.
                if nxt2 is not None:
                    qsteps = qproj_stepper(nxt2[0], nxt2[1])
                    for j, thq in enumerate(qsteps):
                        fill.setdefault(2 * j + 1, []).append(thq)
                    for j, m in enumerate(range(0, 4)):
                        fillB.setdefault(2 + 4 * j, []).append(
                            lambda u=nxt2, mm=m: emit_S_pairs(u[0], u[1],
                                                              [mm]))
                    for j, m in enumerate(range(4, 8)):
                        fillB.setdefault(1 + 4 * j, []).append(
                            lambda u=nxt2, mm=m: emit_S_pairs(u[0], u[1],
                                                              [mm]))
                if nxt2 is None:
                    # final two units have no q-proj/S-firsts to place —
                    # keep the PV-A gaps fed with out-projection drain
                    # steps instead so the PE never idles into the tail
                    for s in (1, 3, 5, 7, 9, 11, 13, 15):
                        fill.setdefault(s, []).append(
                            lambda: outproj_step(1))
                for s in (0, 3, 4, 7, 8, 11, 12, 15):
                    fillB.setdefault(s, []).append(
                        lambda: outproj_step(1))
            if (c, p) == (NCHUNK - 1, NPAIR - 1):
                # pre-accumulate the last chunk's out-projection over pairs
                # 0..2 for e=0..3 while this unit's PV finishes; pair 3 is
                # added after its norm in the tail
                def partial(e):
                    if e % 2 == 0:
                        op3_tiles.append(
                            sps_pool.tile([P, 2 * CH], F32, tag="sps",
                                          name=f"op3_{e}"))
                    half = op3_tiles[e // 2][:, (e % 2) * CH:(e % 2 + 1) * CH]
                    op3_slices[e] = half
                    for pp in range(3):
                        nc.tensor.matmul(
                            half, wo[pp][:, e * P:(e + 1) * P],
                            ot_map[(NCHUNK - 1, pp)][:],
                            start=(pp == 0), stop=False)
                for j in range(4):
                    fillB.setdefault(2 + 4 * j, []).append(
                        lambda e=j: partial(e))
                # the partials need ot of pairs 0..2 — drain the norm queue
                while pend_norm:
                    u = pend_norm.popleft()
                    emit_norm(u)
                    normed[u[0]] += 1

            unit = emit_PV(c, p, fill, fillB,
                           split_evac=((c, p) == (NCHUNK - 1, NPAIR - 1)))
            pend_norm.append(unit)
            # lagged norm (immediate after the last unit)
            while pend_norm and (len(pend_norm) > 1
                                 or (c, p) == (NCHUNK - 1, NPAIR - 1)):
                u = pend_norm.popleft()
                emit_norm(u)
                normed[u[0]] += 1
                if normed[u[0]] == NPAIR and u[0] < NCHUNK - 1:
                    cc = u[0]
                    for e in range(ET):
                        pend_outproj.append((cc, e))

        # tail: any leftover drain blocks, then finish the pre-accumulated
        # e-blocks (pair 3), then the remaining e-blocks pairwise
        # interleaved; stores fan out over all three DMA rings (the scalar
        # ring is free again — the last exp is long gone)
        while pend_outproj or op_active:
            outproj_step(2)
        # the last chunk's ~1MB of stores move as 1KB packets — far below
        # ring burst rate — so fan them out over all three DMA rings (the
        # scalar ring is free again, the last exp is long gone)
        rings = [nc.sync, nc.scalar, nc.gpsimd]
        csl3 = slice((NCHUNK - 1) * CH, NCHUNK * CH)
        # the tail is DVE-latency-bound (every e-block ends in a cast the
        # next PSUM reuse waits on) — alternate the casts between DVE and
        # the now-idle ACT engine (activation Copy) so two run concurrently
        def act_cast(dst, src):
            nc.scalar.activation(dst, src, mybir.ActivationFunctionType.Copy)

        def dve_cast(dst, src):
            nc.vector.tensor_copy(dst, src)

        cast_fns = [dve_cast, act_cast]
        for e in range(4):
            half = op3_slices[e]
            nc.tensor.matmul(half, wo[3][:, e * P:(e + 1) * P],
                             ot_map[(NCHUNK - 1, 3)][:],
                             start=False, stop=True)
            st = stage_pool.tile([P, CH], F16, tag="stg", name=f"st3_{e}")
            with nc.allow_low_precision(reason="fp16 output partials"):
                cast_fns[e % 2](st[:], half)
            rings[e % 3].dma_start(outT_r[e][:, csl3], st[:])
        for e0 in (4, 6):
            # the (3,3) oaug accumulators are already evacuated here, so
            # their pool is free — using it keeps these chains off the
            # aux pool, whose buffers still WAR-wait on the last drain
            # blocks' casts
            pool3 = oaug_pool if e0 == 4 else aux_pool
            tag3 = "oaug" if e0 == 4 else "aux"
            psA = pool3.tile([P, CH], F32, tag=tag3, name=f"pso3_{e0}")
            psB = pool3.tile([P, CH], F32, tag=tag3, name=f"pso3_{e0 + 1}")
            for pp in range(NPAIR):
                nc.tensor.matmul(psA[:], wo[pp][:, e0 * P:(e0 + 1) * P],
                                 ot_map[(NCHUNK - 1, pp)][:],
                                 start=(pp == 0), stop=(pp == NPAIR - 1))
                nc.tensor.matmul(psB[:],
                                 wo[pp][:, (e0 + 1) * P:(e0 + 2) * P],
                                 ot_map[(NCHUNK - 1, pp)][:],
                                 start=(pp == 0), stop=(pp == NPAIR - 1))
            for j, ps in ((0, psA), (1, psB)):
                e = e0 + j
                st = stage_pool.tile([P, CH], F16, tag="stg",
                                     name=f"st3_{e}")
                with nc.allow_low_precision(reason="fp16 output partials"):
                    cast_fns[e % 2](st[:], ps[:])
                rings[e % 3].dma_start(outT_r[e][:, csl3], st[:])

    nc.compile()
    return nc


def _get_nc():
    global _NC_CACHE
    if _NC_CACHE is None:
        _NC_CACHE = _build()
    return _NC_CACHE


def _make_in_maps(x, w_qkv, w_out):
    cst = np.zeros((P, 129), dtype=np.float16)
    cst[:, 0] = 1.0
    cst[0, 1:65] = 1.0
    cst[32, 65:129] = 1.0
    per_g = []
    for g in range(2):
        qk_g = np.concatenate([w_qkv[g * 512:(g + 1) * 512],
                               w_qkv[DI + g * 512:DI + (g + 1) * 512]], axis=0)
        wqkT = np.ascontiguousarray(qk_g.T)               # [1024 d, 1024 f]
        wqkA = np.ascontiguousarray(
            wqkT.reshape(KT, P, 8, P).transpose(2, 1, 0, 3).astype(np.float16))
        v_g = w_qkv[2 * DI + g * 512:2 * DI + (g + 1) * 512]
        wvT = np.ascontiguousarray(v_g.T)                 # [1024 d, 512 f]
        wvA = np.ascontiguousarray(
            wvT.reshape(KT, P, 512).transpose(1, 0, 2).astype(np.float16))
        woTg = np.ascontiguousarray(
            w_out[:, g * 512:(g + 1) * 512].T.astype(np.float16))
        per_g.append((wqkA, wvA, woTg))

    in_maps = []
    for c in range(8):
        b, g = c // 2, c % 2
        wqkA, wvA, woTg = per_g[g]
        in_maps.append({
            "xT": np.ascontiguousarray(x[b].T.astype(np.float16)),
            "wqkA": wqkA,
            "wvA": wvA,
            "woT": woTg,
            "cst": cst,
        })
    return in_maps


def kernel(x, w_qkv, w_out, b_out):
    x = np.asarray(x, dtype=np.float32)
    w_qkv = np.asarray(w_qkv, dtype=np.float32)
    w_out = np.asarray(w_out, dtype=np.float32)
    b_out = np.asarray(b_out, dtype=np.float32)
    B = x.shape[0]

    in_maps = _make_in_maps(x, w_qkv, w_out)
    nc = _get_nc()
    res = run_bass_kernel_spmd(nc, in_maps, core_ids=list(range(8)))
    parts = [r["outT"] for r in res.results]
    out = np.empty((B, N, DI), dtype=np.float32)
    for b in range(B):
        out[b] = (parts[2 * b].astype(np.float32)
                  + parts[2 * b + 1].astype(np.float32)).T + b_out
    return out

